# revision 1
# baseline (speedup 1.0000x reference)
import sys

for _p in ("/opt/trn_rl_repo",):
    if _p not in sys.path:
        sys.path.insert(0, _p)

import numpy as np

B, G, DIM, N = 4, 512, 384, 25088
IMAGE = 224
KS = 8
POOL = IMAGE // KS            # 28
NCORES = 8
HALF = N // 2                 # 12544 points per core
BANDS = 7                     # pool rows per core (56 image rows / 8)
TPB = 14                      # tiles per band
PPT = 128                     # points per tile
BAND_PTS = TPB * PPT          # 1792 = 8 image rows

_CACHE = {}


def _build_program():
    import concourse.mybir as mybir
    from concourse.bacc import Bacc
    from concourse.tile import TileContext
    from concourse.alu_op_type import AluOpType

    f32 = mybir.dt.float32
    f16 = mybir.dt.float16
    u16 = mybir.dt.uint16
    i16 = mybir.dt.int16

    nc = Bacc()

    ptsT_d = nc.dram_tensor("ptsT", [3, HALF], f32, kind="ExternalInput")
    npn_d = nc.dram_tensor("npn", [PPT, BANDS * TPB], f32, kind="ExternalInput")
    cenT_d = nc.dram_tensor("cenT", [3, G], f32, kind="ExternalInput")
    ncn_d = nc.dram_tensor("ncnrep", [PPT, G], f32, kind="ExternalInput")
    feat_d = nc.dram_tensor("featp", [128, 4, DIM], f32, kind="ExternalInput")
    ssel_d = nc.dram_tensor("ssel", [128, 7, POOL], f16, kind="ExternalInput")
    eye_d = nc.dram_tensor("eye28", [POOL, POOL], f32, kind="ExternalInput")
    out_d = nc.dram_tensor("out", [DIM, BANDS * POOL], f32, kind="ExternalOutput")

    with TileContext(nc) as tc:
        with tc.sbuf_pool(name="const", bufs=1) as cpool, \
             tc.sbuf_pool(name="bandio", bufs=2) as bpool, \
             tc.sbuf_pool(name="sel", bufs=2) as spool, \
             tc.sbuf_pool(name="tile", bufs=4) as tpool, \
             tc.sbuf_pool(name="wpool", bufs=3) as wpool, \
             tc.sbuf_pool(name="accout", bufs=1) as apool, \
             tc.sbuf_pool(name="ostage", bufs=2) as opool, \
             tc.psum_pool(name="ps_s", bufs=2) as ps_s_pool, \
             tc.psum_pool(name="ps_a", bufs=2) as ps_a_pool, \
             tc.psum_pool(name="ps_t", bufs=1) as ps_t_pool, \
             tc.psum_pool(name="ps_o", bufs=1) as ps_o_pool:

            cenT = cpool.tile([3, G], f32, name="cenT_sb")
            nc.sync.dma_start(out=cenT, in_=cenT_d[:])
            ncnrep = cpool.tile([PPT, G], f32, name="ncn_sb")
            nc.sync.dma_start(out=ncnrep, in_=ncn_d[:])
            npn = cpool.tile([PPT, BANDS * TPB], f32, name="npn_sb")
            nc.sync.dma_start(out=npn, in_=npn_d[:])
            ssel = cpool.tile([128, 7, POOL], f16, name="ssel_sb")
            feats = cpool.tile([128, 4, DIM], f32, name="feat_sb")
            eye = cpool.tile([POOL, POOL], f32, name="eye_sb")
            atsb = apool.tile([128, 4, BANDS, POOL], f32, name="atsb")

            # software pipeline: selection for band bd, then W/A for band bd-1
            sel_state = {}

            def emit_selection(bd):
                ptsT_b = bpool.tile([3, BAND_PTS], f32, name=f"ptsT_b{bd}", tag="ptsT_b")
                nc.sync.dma_start(
                    out=ptsT_b, in_=ptsT_d[:, bd * BAND_PTS:(bd + 1) * BAND_PTS]
                )

                vband = spool.tile([128, TPB, 8], f32, name=f"vband{bd}", tag="vband")
                iband = spool.tile([128, TPB, 8], u16, name=f"iband{bd}", tag="iband")
                for t in range(TPB):
                    # PSUM = 2*(p.c), bit-exact vs device einsum (rows are 2x,2y,2z)
                    s_ps = ps_s_pool.tile([128, G], f32, name=f"s_ps{bd}_{t}", tag="s_ps")
                    nc.tensor.matmul(
                        out=s_ps,
                        lhsT=ptsT_b[:, t * PPT:(t + 1) * PPT],
                        rhs=cenT,
                        start=True,
                        stop=True,
                    )
                    col = bd * TPB + t
                    # negpncn = fl(-cn - pn) = -fl(pn + cn)  (device add order)
                    npc = tpool.tile([128, G], f32, name=f"npc{bd}_{t}", tag="npc")
                    if t % 2 == 0:
                        nc.gpsimd.tensor_scalar(
                            out=npc,
                            in0=ncnrep,
                            scalar1=npn[:, col:col + 1],
                            scalar2=None,
                            op0=AluOpType.add,
                        )
                    else:
                        nc.scalar.activation(
                            out=npc,
                            in_=ncnrep,
                            func=mybir.ActivationFunctionType.Identity,
                            bias=npn[:, col:col + 1],
                            scale=1.0,
                        )
                    # PSUM -> SBUF on ACT so the add can run on Pool (no PSUM access)
                    ssb = tpool.tile([128, G], f32, name=f"ssb{bd}_{t}", tag="ssb")
                    nc.scalar.copy(out=ssb, in_=s_ps)
                    # sprime = 2mm - fl(pn+cn) = -d2_device bitwise
                    sp = tpool.tile([128, G], f32, name=f"sp{bd}_{t}", tag="sp")
                    nc.gpsimd.tensor_tensor(
                        out=sp, in0=ssb, in1=npc, op=AluOpType.add
                    )
                    nc.vector.max(out=vband[:, t, :], in_=sp)
                    nc.vector.max_index(
                        out=iband[:, t, :], in_max=vband[:, t, :], in_values=sp
                    )

                # weights for the whole band: d2 = max(-sprime_top3, 1e-10)
                d2 = spool.tile([128, TPB, 3], f32, name=f"d2{bd}", tag="d2")
                nc.gpsimd.tensor_scalar(
                    out=d2,
                    in0=vband[:, :, 0:3],
                    scalar1=-1.0,
                    scalar2=1e-10,
                    op0=AluOpType.mult,
                    op1=AluOpType.max,
                )
                rec = spool.tile([128, TPB, 3], f32, name=f"rec{bd}", tag="rec")
                nc.vector.reciprocal(out=rec, in_=d2)
                rsum = spool.tile([128, TPB, 1], f32, name=f"rsum{bd}", tag="rsum")
                nc.vector.tensor_reduce(
                    out=rsum[:, :, 0], in_=rec, axis=mybir.AxisListType.X, op=AluOpType.add
                )
                rinv = spool.tile([128, TPB, 1], f32, name=f"rinv{bd}", tag="rinv")
                nc.vector.reciprocal(out=rinv, in_=rsum)
                w4 = spool.tile([128, TPB, 4], f16, name=f"w4{bd}", tag="w4")
                nc.gpsimd.memset(w4, 0)
                nc.gpsimd.tensor_tensor(
                    out=w4[:, :, 0:3],
                    in0=rec,
                    in1=rinv.broadcast_to([128, TPB, 3]),
                    op=AluOpType.mult,
                )
                i4 = spool.tile([128, TPB, 4], i16, name=f"i4{bd}", tag="i4")
                nc.gpsimd.memset(i4, -1)
                nc.gpsimd.tensor_copy(out=i4[:, :, 0:3], in_=iband[:, :, 0:3].bitcast(i16))
                sel_state[bd] = (w4, i4)

            def emit_scatter_accum(bd):
                w4, i4 = sel_state.pop(bd)
                a_ps = ps_a_pool.tile([POOL, G], f32, name=f"a_ps{bd}", tag="a_ps")
                for t in range(TPB):
                    wt = wpool.tile([128, G], f16, name=f"wt{bd}_{t}", tag="wt")
                    nc.gpsimd.local_scatter(
                        out_ap=wt,
                        data_ap=w4[:, t, :],
                        idxs_ap=i4[:, t, :],
                        channels=128,
                        num_elems=G,
                        num_idxs=4,
                    )
                    nc.tensor.matmul(
                        out=a_ps,
                        lhsT=ssel[:, t % 7, :],
                        rhs=wt,
                        start=(t == 0),
                        stop=(t == TPB - 1),
                    )
                acp = spool.tile([POOL, G], f32, name=f"acp{bd}", tag="acp")
                nc.scalar.copy(out=acp, in_=a_ps)
                for c in range(4):
                    t_ps = ps_t_pool.tile([128, POOL], f32, name=f"t_ps{bd}_{c}", tag="t_ps")
                    nc.tensor.transpose(
                        out=t_ps, in_=acp[:, c * 128:(c + 1) * 128], identity=eye
                    )
                    nc.scalar.copy(out=atsb[:, c, bd, :], in_=t_ps)
                # fold finals per band: pooled[d, bd*28+pc] = sum_g feat[g,d]*AT[g,bd,pc]
                for dc in range(3):
                    for gc in range(4):
                        nc.tensor.matmul(
                            out=o_ps[dc][:, bd * POOL:(bd + 1) * POOL],
                            lhsT=feats[:, gc, dc * 128:(dc + 1) * 128],
                            rhs=atsb[:, gc, bd, :],
                            start=(gc == 0),
                            stop=(gc == 3),
                        )

            o_ps = [
                ps_o_pool.tile([128, BANDS * POOL], f32, name=f"o_ps{dc}", tag=f"o_ps{dc}")
                for dc in range(3)
            ]
            for bd in range(BANDS + 1):
                if bd < BANDS:
                    emit_selection(bd)
                if bd == 1:
                    # deferred const loads: needed first by scatter (ssel/eye)
                    # and finals (feats) — keep them off band-0's critical path
                    nc.sync.dma_start(out=ssel, in_=ssel_d[:])
                    nc.sync.dma_start(out=eye, in_=eye_d[:])
                    nc.sync.dma_start(out=feats, in_=feat_d[:])
                if bd >= 1:
                    emit_scatter_accum(bd - 1)

            for dc in range(3):
                osb = opool.tile([128, BANDS * POOL], f32, name=f"osb{dc}", tag="osb")
                nc.scalar.copy(out=osb, in_=o_ps[dc])
                nc.sync.dma_start(out=out_d[dc * 128:(dc + 1) * 128, :], in_=osb)

    nc.finalize()
    return nc


def _host_inputs(group_features, group_centers, original_points, core):
    b, h = core // 2, core % 2
    pts = np.asarray(original_points[b, h * HALF:(h + 1) * HALF], dtype=np.float32)

    ptsT = np.ascontiguousarray((2.0 * pts).T)              # (3, HALF) rows 2x,2y,2z

    # pn in device add order: (x^2 + y^2) + z^2, fp32
    pn = (pts[:, 0] * pts[:, 0] + pts[:, 1] * pts[:, 1]) + pts[:, 2] * pts[:, 2]
    # npn[p, bd*TPB+t] = -pn[bd*1792 + t*128 + p]
    npn = np.ascontiguousarray(
        (-pn).reshape(BANDS, TPB, PPT).transpose(2, 0, 1).reshape(PPT, BANDS * TPB)
    )

    cen = np.asarray(group_centers[b], dtype=np.float32)   # (512, 3)
    cenT = np.ascontiguousarray(cen.T)                     # (3, G)
    cn = (cen[:, 0] * cen[:, 0] + cen[:, 1] * cen[:, 1]) + cen[:, 2] * cen[:, 2]
    ncnrep = np.ascontiguousarray(np.tile(-cn[None, :], (PPT, 1)))

    feat = np.asarray(group_features[b], dtype=np.float32)  # (512, 384)
    featp = np.ascontiguousarray(feat.reshape(4, 128, DIM).transpose(1, 0, 2))

    return {
        "ptsT": ptsT,
        "npn": npn,
        "cenT": cenT,
        "ncnrep": ncnrep,
        "featp": featp,
        "ssel": _ssel(),
        "eye28": np.eye(POOL, dtype=np.float32),
    }


def _ssel():
    s = np.zeros((128, 7, POOL), dtype=np.float16)
    for phi in range(7):
        for p in range(128):
            pc = ((phi * 128 + p) % IMAGE) // KS
            s[p, phi, pc] = 1.0 / 64.0
    return s


def _numpy_fallback(group_features, group_centers, original_points, nonzero_indices, kernel_size):
    gf = np.asarray(group_features, dtype=np.float64)
    cen = np.asarray(group_centers, dtype=np.float64)
    pts = np.asarray(original_points, dtype=np.float64)
    ks = int(kernel_size)
    out = np.zeros((B, DIM, IMAGE * IMAGE), dtype=np.float64)
    for b in range(B):
        d2 = (
            np.sum(pts[b] ** 2, axis=1)[:, None]
            + np.sum(cen[b] ** 2, axis=1)[None, :]
            - 2.0 * pts[b] @ cen[b].T
        )
        idx = np.argsort(d2, axis=1)[:, :3]
        d = np.maximum(np.take_along_axis(d2, idx, axis=1), 1e-10)
        rec = 1.0 / d
        w = rec / rec.sum(axis=1, keepdims=True)
        interp = np.einsum("nkd,nk->dn", gf[b][idx], w)
        out[b][:, np.asarray(nonzero_indices)] = interp
    ho = IMAGE // ks
    pooled = out.reshape(B, DIM, ho, ks, ho, ks).mean(axis=(3, 5))
    return pooled.astype(np.float32)


def kernel(group_features, group_centers, original_points, nonzero_indices, kernel_size):
    nz = np.asarray(nonzero_indices)
    ks = int(np.asarray(kernel_size))
    if ks != KS or nz.shape != (N,) or not np.array_equal(nz, np.arange(N)):
        return _numpy_fallback(
            group_features, group_centers, original_points, nonzero_indices, kernel_size
        )

    from concourse.bass_utils import run_bass_kernel_spmd

    if "nc" not in _CACHE:
        _CACHE["nc"] = _build_program()
    nc = _CACHE["nc"]

    in_maps = [
        _host_inputs(group_features, group_centers, original_points, c)
        for c in range(NCORES)
    ]
    res = run_bass_kernel_spmd(nc, in_maps, core_ids=list(range(NCORES))).results

    out = np.zeros((B, DIM, POOL, POOL), dtype=np.float32)
    for c in range(NCORES):
        b, h = c // 2, c % 2
        out[b, :, 7 * h:7 * h + 7, :] = res[c]["out"].reshape(DIM, BANDS, POOL)
    return out



# revision 10
# speedup vs baseline: 2.8795x; 2.8795x over previous
import sys

for _p in ("/opt/trn_rl_repo",):
    if _p not in sys.path:
        sys.path.insert(0, _p)

import numpy as np

B, G, DIM, N = 4, 512, 384, 25088
IMAGE = 224
KS = 8
POOL = IMAGE // KS            # 28
NCORES = 8
HALF = N // 2                 # 12544 points per core
PPT = 128                     # points per tile
NT = HALF // PPT              # 98 normal tiles
CELLS = 7 * POOL              # 196 pool cells per core
TB = 14                       # tiles per weight-math batch
GRIDB = 16                    # host pruning grid (16^3)
WIDE_PT = 128                 # point cand-range width beyond which -> wide tile
SLAB = 256                    # max normal window width
K = 24                        # contraction rows (bf16 3-way split)

_CACHE = {}


# ---------------------------------------------------------------- host: plan

def _morton_order(cen, bits=6):
    g = np.clip((cen * (1 << bits)).astype(np.int64), 0, (1 << bits) - 1)
    h = np.zeros(cen.shape[0], dtype=np.int64)
    for b in range(bits - 1, -1, -1):
        for i in range(3):
            h = (h << 1) | ((g[:, i] >> b) & 1)
    return np.argsort(h, kind="stable")


def _cell_ranges(cen_s):
    """Conservative per-grid-cell candidate id range [a,b] (inclusive)."""
    ncell = GRIDB
    cs = 1.0 / ncell
    halfdiag = cs * np.sqrt(3.0) / 2.0
    ax = (np.arange(ncell) + 0.5) * cs
    cc = np.stack(np.meshgrid(ax, ax, ax, indexing="ij"), axis=-1).reshape(-1, 3)
    d = np.linalg.norm(
        cc[:, None, :].astype(np.float32) - cen_s[None, :, :].astype(np.float32),
        axis=2,
    )
    r3 = np.partition(d, 2, axis=1)[:, 2]
    mask = d <= (r3 + 2.0 * halfdiag)[:, None]
    gids = np.arange(G)
    a = np.where(mask, gids[None, :], G).min(axis=1)
    b = np.where(mask, gids[None, :], -1).max(axis=1)
    return a, b


def _split3(v):
    """3-way bf16 split of float64 array -> (hi, mid, lo) as float64 values."""
    from ml_dtypes import bfloat16

    h = v.astype(bfloat16).astype(np.float64)
    m = (v - h).astype(bfloat16).astype(np.float64)
    l = (v - h - m).astype(bfloat16).astype(np.float64)
    return h, m, l


def _plan(group_centers, original_points):
    """Build the shared SPMD tile plan + per-core assignments."""
    centers = np.asarray(group_centers, dtype=np.float64)
    points = np.asarray(original_points, dtype=np.float64)

    batch = []
    for b in range(B):
        order = _morton_order(centers[b])
        cen_s = centers[b][order]
        a, bb = _cell_ranges(cen_s)
        batch.append((order, cen_s, a, bb))

    cores = []
    max_wide = 0
    for c in range(NCORES):
        b, h = c // 2, c % 2
        order, cen_s, a, bb = batch[b]
        pts = points[b, h * HALF:(h + 1) * HALF]
        ci = (
            np.clip((pts * GRIDB).astype(np.int64), 0, GRIDB - 1)
            @ np.array([GRIDB * GRIDB, GRIDB, 1])
        )
        pa, pb = a[ci], bb[ci]
        width = pb - pa + 1
        wide = width > WIDE_PT
        n_wide = int(wide.sum())
        max_wide = max(max_wide, n_wide)
        nrm = np.nonzero(~wide)[0]
        mid = (pa[nrm] + pb[nrm]).astype(np.float64) * 0.5
        nrm = nrm[np.argsort(mid, kind="stable")]
        cores.append({"pa": pa, "pb": pb, "nrm": nrm, "wide": np.nonzero(wide)[0]})

    n_wide_tiles = (max_wide + PPT - 1) // PPT
    T = NT + n_wide_tiles

    # per-core per-tile source points (local index or -1 = pad)
    for core in cores:
        src = np.full((T, PPT), -1, dtype=np.int64)
        nrm = core["nrm"]
        # distribute normal points over NT tiles: tile t gets slots so that
        # every tile has <= PPT; short cores pad at the end of each tile
        per = len(nrm) // NT
        extra = len(nrm) - per * NT
        pos = 0
        for t in range(NT):
            k = per + (1 if t < extra else 0)
            src[t, :k] = nrm[pos:pos + k]
            pos += k
        w = core["wide"]
        for j, t in enumerate(range(NT, T)):
            seg = w[j * PPT:(j + 1) * PPT]
            src[t, :len(seg)] = seg
        core["src"] = src

    # program windows: union over cores, 32-aligned
    lo = np.zeros(T, dtype=np.int64)
    hi = np.full(T, G, dtype=np.int64)
    for t in range(NT):
        amin, bmax = G, -1
        for core in cores:
            s = core["src"][t]
            s = s[s >= 0]
            if len(s):
                amin = min(amin, core["pa"][s].min())
                bmax = max(bmax, core["pb"][s].max())
        if bmax < 0:
            amin, bmax = 0, 31
        l = (amin // 32) * 32
        hh = min(G, ((bmax + 1 + 31) // 32) * 32)
        if hh - l > SLAB:
            l, hh = 0, G  # fallback: full scan for this tile
        lo[t], hi[t] = l, hh
    # wide tiles scan everything
    for t in range(NT, T):
        lo[t], hi[t] = 0, G

    return {"T": T, "lo": lo, "hi": hi, "batch": batch, "cores": cores}


def _legal_pieces(lo, hi):
    """Split window [lo,hi) into matmul-legal (group64, ppos, m, wtoff) pieces.

    W^T accumulates in 8 psum tiles of (64, CELLS); legal out partition
    offsets within a tile are 0 (m<=64) and 32 (m<=32)."""
    pieces = []
    x = lo
    while x < hi:
        g = x // 64
        gend = min(hi, (g + 1) * 64)
        p0 = x - g * 64
        m = (gend - x) if p0 == 0 else min(32, gend - x)
        pieces.append((g, p0, m, x - lo))
        x += m
    return pieces


# ------------------------------------------------------------ device program

def _build_program(plan):
    import concourse.mybir as mybir
    from concourse.bacc import Bacc
    from concourse.tile import TileContext
    from concourse.alu_op_type import AluOpType

    f32 = mybir.dt.float32
    f16 = mybir.dt.float16
    bf16 = mybir.dt.bfloat16
    u16 = mybir.dt.uint16
    i16 = mybir.dt.int16

    T = plan["T"]
    lo, hi = plan["lo"], plan["hi"]
    W = [int(hi[t] - lo[t]) for t in range(T)]

    nc = Bacc()

    ptsT_d = nc.dram_tensor("ptsT", [K, T * PPT], bf16, kind="ExternalInput")
    cenT_d = nc.dram_tensor("cenT", [K, G], bf16, kind="ExternalInput")
    sel_d = nc.dram_tensor("sel", [128, T, CELLS], f16, kind="ExternalInput")
    feat_d = nc.dram_tensor("featp", [64, 8, DIM], f16, kind="ExternalInput")
    zc_d = nc.dram_tensor("zc", [1, 512], f16, kind="ExternalInput")
    out_d = nc.dram_tensor("out", [DIM, CELLS], f32, kind="ExternalOutput")

    batches = [list(range(i, min(i + TB, T))) for i in range(0, T, TB)]

    # per-chunk last contribution (for stop flags)
    chunk_tiles = {c: [] for c in range(8)}
    for t in range(T):
        for (c, p0, m, off) in _legal_pieces(int(lo[t]), int(hi[t])):
            chunk_tiles[c].append(t)
    chunk_last = {c: (v[-1] if v else -1) for c, v in chunk_tiles.items()}

    with TileContext(nc) as tc:
        with tc.sbuf_pool(name="const", bufs=1) as cpool, \
             tc.sbuf_pool(name="big", bufs=1) as gpool, \
             tc.sbuf_pool(name="ssb", bufs=4) as spool, \
             tc.sbuf_pool(name="ssbw", bufs=2) as swpool, \
             tc.sbuf_pool(name="band", bufs=2) as vpool, \
             tc.sbuf_pool(name="wt", bufs=3) as wpool, \
             tc.sbuf_pool(name="wtw", bufs=2) as wwpool, \
             tc.sbuf_pool(name="fin", bufs=1) as fpool, \
             tc.psum_pool(name="ps_s", bufs=2) as ps_s, \
             tc.psum_pool(name="ps_wt", bufs=1) as ps_wt, \
             tc.psum_pool(name="ps_o", bufs=1) as ps_o:

            cenT = cpool.tile([K, G], bf16, name="cenT_sb")
            nc.sync.dma_start(out=cenT, in_=cenT_d[:])
            zc = cpool.tile([1, 512], f16, name="zc_sb")
            nc.sync.dma_start(out=zc, in_=zc_d[:])
            feats = cpool.tile([64, 8, DIM], f16, name="feat_sb")
            nc.sync.dma_start(out=feats, in_=feat_d[:])

            ptsT = gpool.tile([K, T * PPT], bf16, name="ptsT_sb")
            sel = gpool.tile([128, T, CELLS], f16, name="sel_sb")
            # chunked loads so early tiles' data arrives first
            nch = 4
            for i in range(nch):
                t0 = (T * i) // nch
                t1 = (T * (i + 1)) // nch
                nc.sync.dma_start(
                    out=ptsT[:, t0 * PPT:t1 * PPT], in_=ptsT_d[:, t0 * PPT:t1 * PPT]
                )
                nc.sync.dma_start(out=sel[:, t0:t1, :], in_=sel_d[:, t0:t1, :])

            # W^T accumulators (centers chunk-major), zero-initialized
            # each pair tile spans exactly 2048B/partition so that psum
            # zero-region bookkeeping lines up with partition-offset writes
            wt_pairs = [
                ps_wt.tile([64, 512], f32, name=f"wt_ps{c}", tag=f"wt_ps{c}")
                for c in range(4)
            ]
            wt_ps = [
                wt_pairs[c // 2][:, (c % 2) * 256:(c % 2) * 256 + CELLS]
                for c in range(8)
            ]
            for c in range(4):
                nc.tensor.matmul(
                    out=wt_pairs[c],
                    lhsT=zc[:, 0:64],
                    rhs=zc[:, 0:512],
                    start=True,
                    stop=False,
                    skip_group_check=True,
                )

            state = {}

            def emit_sel_batch(bi):
                tiles = batches[bi]
                nb = len(tiles)
                vband = vpool.tile([128, nb, 8], f32, name=f"vb{bi}", tag="vb")
                iband = vpool.tile([128, nb, 8], u16, name=f"ib{bi}", tag="ib")
                for j, t in enumerate(tiles):
                    w = W[t]
                    if w > SLAB:
                        ssb = swpool.tile([128, G], f32, name=f"ssw{t}", tag="ssw")
                        for half in range(2):
                            s_ps = ps_s.tile(
                                [128, SLAB], f32, name=f"sp{t}_{half}", tag="sp"
                            )
                            nc.tensor.matmul(
                                out=s_ps,
                                lhsT=ptsT[:, t * PPT:(t + 1) * PPT],
                                rhs=cenT[:, half * SLAB:(half + 1) * SLAB],
                                start=True,
                                stop=True,
                            )
                            nc.scalar.copy(
                                out=ssb[:, half * SLAB:(half + 1) * SLAB], in_=s_ps
                            )
                    else:
                        s_ps_full = ps_s.tile([128, SLAB], f32, name=f"sp{t}", tag="sp")
                        s_ps = s_ps_full[:, 0:w]
                        ssb_full = spool.tile([128, SLAB], f32, name=f"ss{t}", tag="ss")
                        ssb = ssb_full[:, 0:w]
                        nc.tensor.matmul(
                            out=s_ps,
                            lhsT=ptsT[:, t * PPT:(t + 1) * PPT],
                            rhs=cenT[:, lo[t]:hi[t]],
                            start=True,
                            stop=True,
                        )
                        nc.scalar.copy(out=ssb, in_=s_ps)
                    nc.vector.max(out=vband[:, j, :], in_=ssb)
                    nc.vector.max_index(
                        out=iband[:, j, :], in_max=vband[:, j, :], in_values=ssb
                    )
                # weights for the whole batch
                d2 = vpool.tile([128, nb, 3], f32, name=f"d2{bi}", tag="d2")
                nc.gpsimd.tensor_scalar(
                    out=d2,
                    in0=vband[:, :, 0:3],
                    scalar1=-1.0,
                    scalar2=1e-10,
                    op0=AluOpType.mult,
                    op1=AluOpType.max,
                )
                rec = vpool.tile([128, nb, 3], f32, name=f"rc{bi}", tag="rc")
                nc.vector.reciprocal(out=rec, in_=d2)
                rsum = vpool.tile([128, nb, 1], f32, name=f"rs{bi}", tag="rs")
                nc.vector.tensor_reduce(
                    out=rsum[:, :, 0], in_=rec, axis=mybir.AxisListType.X,
                    op=AluOpType.add,
                )
                rinv = vpool.tile([128, nb, 1], f32, name=f"ri{bi}", tag="ri")
                nc.vector.reciprocal(out=rinv, in_=rsum)
                w4 = vpool.tile([128, nb, 4], f16, name=f"w4{bi}", tag="w4")
                nc.gpsimd.memset(w4, 0)
                nc.gpsimd.tensor_tensor(
                    out=w4[:, :, 0:3],
                    in0=rec,
                    in1=rinv.broadcast_to([128, nb, 3]),
                    op=AluOpType.mult,
                )
                i4 = vpool.tile([128, nb, 4], i16, name=f"i4{bi}", tag="i4")
                nc.gpsimd.memset(i4, -1)
                nc.gpsimd.tensor_copy(out=i4[:, :, 0:3], in_=iband[:, :, 0:3].bitcast(i16))
                state[bi] = (w4, i4)

            def emit_scatter_batch(bi):
                tiles = batches[bi]
                w4, i4 = state.pop(bi)
                for j, t in enumerate(tiles):
                    w = W[t]
                    if w > SLAB:
                        wt = wwpool.tile([128, G], f16, name=f"wtw{t}", tag="wtw")
                        nelem = G
                    else:
                        wt_full = wpool.tile([128, SLAB], f16, name=f"wt{t}", tag="wt")
                        wt = wt_full[:, 0:w]
                        nelem = w
                    nc.gpsimd.local_scatter(
                        out_ap=wt,
                        data_ap=w4[:, j, :],
                        idxs_ap=i4[:, j, :],
                        channels=128,
                        num_elems=nelem,
                        num_idxs=4,
                    )
                    for (c, p0, m, off) in _legal_pieces(int(lo[t]), int(hi[t])):
                        nc.tensor.matmul(
                            out=wt_ps[c][p0:p0 + m, :],
                            lhsT=wt[:, off:off + m],
                            rhs=sel[:, t, :],
                            start=False,
                            stop=(t == chunk_last[c]),
                            skip_group_check=True,
                        )

            prev = None
            for bi in range(len(batches)):
                emit_sel_batch(bi)
                if prev is not None:
                    emit_scatter_batch(prev)
                prev = bi
            emit_scatter_batch(prev)

            # finals: pooled = feat^T @ W  (per 128-dim chunk)
            wtsb = []
            for c in range(8):
                wsb = fpool.tile([64, CELLS], f16, name=f"wsb{c}")
                nc.scalar.copy(out=wsb, in_=wt_ps[c])
                wtsb.append(wsb)
            o_pair = ps_o.tile([128, 2, CELLS], f32, name="o_pair", tag="o_pair")
            o_last = ps_o.tile([128, CELLS], f32, name="o_last", tag="o_last")
            nc.tensor.matmul(
                out=o_pair,
                lhsT=zc[:, 0:128],
                rhs=zc[:, 0:2 * CELLS],
                start=True,
                stop=False,
                skip_group_check=True,
            )
            for dc in range(3):
                o_ps = o_pair[:, dc, :] if dc < 2 else o_last
                for c in range(8):
                    nc.tensor.matmul(
                        out=o_ps,
                        lhsT=feats[:, c, dc * 128:(dc + 1) * 128],
                        rhs=wtsb[c],
                        start=(c == 0 and dc == 2),
                        stop=(c == 7),
                        skip_group_check=True,
                    )
                osb = fpool.tile([128, CELLS], f32, name=f"osb{dc}")
                nc.scalar.copy(out=osb, in_=o_ps)
                nc.sync.dma_start(out=out_d[dc * 128:(dc + 1) * 128, :], in_=osb)

    nc.finalize()
    return nc


# ------------------------------------------------------------- host: inputs

def _host_inputs(plan, group_features, group_centers, original_points, core):
    from ml_dtypes import bfloat16

    b, h = core // 2, core % 2
    T = plan["T"]
    order, cen_s, _, _ = plan["batch"][b]
    src = plan["cores"][core]["src"]          # (T, PPT) local point idx or -1
    lo = plan["lo"]

    pts = np.asarray(
        original_points[b, h * HALF:(h + 1) * HALF], dtype=np.float64
    )

    # gather per-tile points (pads use point 0)
    psrc = np.where(src >= 0, src, 0).reshape(-1)          # (T*PPT,)
    p = pts[psrc]                                           # (T*PPT, 3)
    pn = (p * p).sum(axis=1)

    # product expansion: v*v' ~= hh' + hm' + mh' + hl' + lh' + mm'
    PAIRS = [(0, 0), (0, 1), (1, 0), (0, 2), (2, 0), (1, 1)]
    ptsT = np.zeros((K, T * PPT), dtype=np.float64)
    for i in range(3):
        sp = _split3(2.0 * p[:, i])
        for j, (a, _) in enumerate(PAIRS):
            ptsT[6 * i + j] = sp[a]
    hh, mm, ll = _split3(pn)
    ptsT[18], ptsT[19], ptsT[20] = hh, mm, ll
    ptsT[21] = ptsT[22] = ptsT[23] = 1.0

    cn = (cen_s * cen_s).sum(axis=1)
    PAIRS = [(0, 0), (0, 1), (1, 0), (0, 2), (2, 0), (1, 1)]
    cenT = np.zeros((K, G), dtype=np.float64)
    for i in range(3):
        sp = _split3(cen_s[:, i])
        for j, (_, a) in enumerate(PAIRS):
            cenT[6 * i + j] = sp[a]
    cenT[18] = cenT[19] = cenT[20] = -1.0
    hh, mm, ll = _split3(cn)
    cenT[21], cenT[22], cenT[23] = -hh, -mm, -ll

    # cell one-hot (1/64 folded in), pads -> zero row
    sel = np.zeros((128, T, CELLS), dtype=np.float16)
    gidx = h * HALF + psrc                                  # global point index
    row = gidx // IMAGE
    col = gidx % IMAGE
    cell = (row // KS - 7 * h) * POOL + col // KS           # (T*PPT,)
    tt = np.repeat(np.arange(T), PPT)
    pp = np.tile(np.arange(PPT), T)
    valid = src.reshape(-1) >= 0
    sel[pp[valid], tt[valid], cell[valid]] = 1.0 / 64.0

    feat = np.asarray(group_features[b], dtype=np.float32)[order]   # sorted
    featp = np.ascontiguousarray(
        feat.reshape(8, 64, DIM).transpose(1, 0, 2)
    ).astype(np.float16)

    return {
        "ptsT": ptsT.astype(bfloat16),
        "cenT": cenT.astype(bfloat16),
        "sel": sel,
        "featp": featp,
        "zc": np.zeros((1, 512), dtype=np.float16),
    }


# ------------------------------------------------------------------ fallback

def _numpy_fallback(group_features, group_centers, original_points,
                    nonzero_indices, kernel_size):
    gf = np.asarray(group_features, dtype=np.float64)
    cen = np.asarray(group_centers, dtype=np.float64)
    pts = np.asarray(original_points, dtype=np.float64)
    ks = int(kernel_size)
    out = np.zeros((B, DIM, IMAGE * IMAGE), dtype=np.float64)
    for b in range(B):
        d2 = (
            np.sum(pts[b] ** 2, axis=1)[:, None]
            + np.sum(cen[b] ** 2, axis=1)[None, :]
            - 2.0 * pts[b] @ cen[b].T
        )
        idx = np.argsort(d2, axis=1)[:, :3]
        d = np.maximum(np.take_along_axis(d2, idx, axis=1), 1e-10)
        rec = 1.0 / d
        w = rec / rec.sum(axis=1, keepdims=True)
        interp = np.einsum("nkd,nk->dn", gf[b][idx], w)
        out[b][:, np.asarray(nonzero_indices)] = interp
    ho = IMAGE // ks
    pooled = out.reshape(B, DIM, ho, ks, ho, ks).mean(axis=(3, 5))
    return pooled.astype(np.float32)


# -------------------------------------------------------------------- kernel

def kernel(group_features, group_centers, original_points, nonzero_indices,
           kernel_size):
    nz = np.asarray(nonzero_indices)
    ks = int(np.asarray(kernel_size))
    if ks != KS or nz.shape != (N,) or not np.array_equal(nz, np.arange(N)):
        return _numpy_fallback(
            group_features, group_centers, original_points, nonzero_indices,
            kernel_size,
        )

    from concourse.bass_utils import run_bass_kernel_spmd

    plan = _plan(group_centers, original_points)
    nc = _build_program(plan)
    _CACHE["nc"] = nc
    _CACHE["plan"] = plan

    in_maps = [
        _host_inputs(plan, group_features, group_centers, original_points, c)
        for c in range(NCORES)
    ]
    res = run_bass_kernel_spmd(nc, in_maps, core_ids=list(range(NCORES))).results

    out = np.zeros((B, DIM, POOL, POOL), dtype=np.float32)
    for c in range(NCORES):
        b, h = c // 2, c % 2
        out[b, :, 7 * h:7 * h + 7, :] = res[c]["out"].reshape(DIM, 7, POOL)
    return out
